# revision 12
# baseline (speedup 1.0000x reference)
"""Trainium2 Bass kernel for nn_Block_35837207118566 (IBP causal attention block).

Key structural insight: setup_inputs uses x_lower = x - eps, x_upper = x + eps
with a SCALAR eps, so the interval radius d = (x_upper-x_lower)/2 is constant.
Then qd = d0*rowsum|Wq| etc. are position-independent vectors, and:

  S_ll = A - u[t] - w[s] + c      (A = exact scores, u/w rank-1 corrections)
  S_lu = A + u[t] - w[s] - c
  S_ul = A - u[t] + w[s] - c
  S_uu = A + u[t] + w[s] + c

Per-row-constant terms (u[t], c) cancel in softmax, so only TWO distinct
interval prob matrices remain: P_- = softmax(A - w[s]) and P_+ = softmax(A + w[s]).
Further, P@v_lower = P@v_mid - vd and P@v_upper = P@v_mid + vd (rows sum to 1,
vd >= 0 constant), so

  y_lower = min(P_-@v, P_+@v) - vd,   y_upper = max(P_-@v, P_+@v) + vd

and the +-vd shift commutes with Wproj (host adds vd@Wproj.T once).
Finally exp(A -+ w) = exp(A)*e^{-+w}: fold e^{-+w[s]} into per-key-scaled V
slabs so ONE exp matrix serves all three attention products, with softmax
denominators from appended e^{-+w} columns.

On-device per core (batch b x head-group g, 6 heads each):
  stage 1: q,k,v projections (m @ W.T only; k pre-scaled by 1/sqrt(D))
  stage 2: per head: S^T = K^T blocks x Q^T (exact only), exp, causal mask,
           then A@[v|1|v*e^-w|e^-w|v*e^+w|e^+w] (195 cols), normalize, min/max
  stage 3: transpose y slabs, partial @ Wproj.T, DMA out; host sums 2 cores
           per batch and applies -+ vd@Wproj.T.
"""

import numpy as np
import ml_dtypes
from contextlib import ExitStack

import concourse.bass as bass
import concourse.bacc as bacc
import concourse.tile as tile
from concourse import mybir
from concourse.masks import make_identity, make_upper_triangular

BF16 = mybir.dt.bfloat16
F32 = mybir.dt.float32
bfloat16 = ml_dtypes.bfloat16
MULT = mybir.AluOpType.mult
MIN = mybir.AluOpType.min
MAX = mybir.AluOpType.max
EXP = mybir.ActivationFunctionType.Exp

B, T, C = 4, 1024, 768
H, D = 12, 64
G = 2                 # head groups (cores per batch)
HPG = H // G          # 6 heads per group
DG = HPG * D          # 384
CT = C // 128         # 6 contraction tiles
TT = T // 128         # 8 sequence tiles
MT = DG // 128        # 3 partition tiles per q/k slab
N_CORES = 8
VW = 195              # v-slab cols per head: [v|1|v*e^-w|e^-w|v*e^+w|e^+w]


def _ap(t, off, dims):
    return bass.AP(tensor=t.tensor, offset=t.offset + off, ap=t.ap[:1] + dims)


def _body(tc, reps=1):
    nc = tc.nc
    mT = nc.dram_tensor("mT", [C, T], BF16, kind="ExternalInput").ap()
    wg = nc.dram_tensor("wg", [C, 3 * DG], BF16, kind="ExternalInput").ap()
    wpT = nc.dram_tensor("wpT", [DG, C], BF16, kind="ExternalInput").ap()
    ew = nc.dram_tensor("ew", [T, 2 * HPG], F32, kind="ExternalInput").ap()

    def _once(rep):
        sfx = "" if reps == 1 else str(rep)
        oy = nc.dram_tensor("oy" + sfx, [T, C], BF16, kind="ExternalOutput").ap()
        ol = nc.dram_tensor("ol" + sfx, [T, C], BF16, kind="ExternalOutput").ap()
        ou = nc.dram_tensor("ou" + sfx, [T, C], BF16, kind="ExternalOutput").ap()
        with ExitStack() as ctx:
            persist = ctx.enter_context(tc.tile_pool(name="persist", bufs=1))

            qk = {nm: persist.tile([128, MT, T], BF16, tag=nm, name=nm)
                  for nm in ("qe", "ke")}
            v3 = persist.tile([128, TT, HPG * VW], BF16, tag="v3")
            v3v = v3.rearrange("p t (h c) -> p t h c", c=VW)
            ysl = {nm: persist.tile([128, TT, DG], BF16, tag=nm, name=nm)
                   for nm in ("ye", "yl", "yu")}
            wps = persist.tile([128, MT, C], BF16, tag="wps")
            msk = persist.tile([128, 128], BF16, tag="msk")
            make_upper_triangular(nc, msk, val=1.0, diag=True)
            ident = persist.tile([128, 128], BF16, tag="ident")
            make_identity(nc, ident)

            # ---------------- stage 1: projections ----------------
            with tc.tile_pool(name="s1src", bufs=1) as s1src, \
                 tc.tile_pool(name="s1ps", bufs=2, space="PSUM") as s1ps:
                ms = s1src.tile([128, CT, T], BF16, tag="ms")
                wgs = s1src.tile([128, CT, 3 * DG], BF16, tag="wgs")
                ewsb = s1src.tile([128, TT, 2 * HPG], F32, tag="ewsb")
                # chunked input DMAs so the first matmuls start early
                mT_r = mT.rearrange("(a p) t -> p a t", p=128)
                wg_r = wg.rearrange("(a p) c -> p a c", p=128)
                nc.sync.dma_start(ms[:, :, 0:512], mT_r[:, :, 0:512])
                nc.sync.dma_start(wgs[:, :, 0:DG], wg_r[:, :, 0:DG])
                nc.sync.dma_start(ms[:, :, 512:T], mT_r[:, :, 512:T])
                nc.sync.dma_start(wgs[:, :, DG:2 * DG], wg_r[:, :, DG:2 * DG])
                nc.sync.dma_start(wgs[:, :, 2 * DG:3 * DG], wg_r[:, :, 2 * DG:3 * DG])
                nc.sync.dma_start(ewsb, ew.rearrange("(a p) x -> p a x", p=128))

                # transposed q/k slabs (exact path only)
                for wofs, nm in ((0, "qe"), (DG, "ke")):
                    for mt in range(MT):
                        for n0 in range(0, T, 512):
                            ps = s1ps.tile([128, 512], F32, tag="psA")
                            for kt in range(CT):
                                nc.tensor.matmul(
                                    ps, lhsT=wgs[:, kt, wofs + mt * 128:wofs + mt * 128 + 128],
                                    rhs=ms[:, kt, n0:n0 + 512],
                                    start=(kt == 0), stop=(kt == CT - 1))
                            nc.scalar.copy(qk[nm][:, mt, n0:n0 + 512], ps)

                # v slab: per head [v | 1 | v*e^-w | e^-w | v*e^+w | e^+w]
                nc.vector.memset(v3v[:, :, :, 64:65], 1.0)
                for tt in range(TT):
                    psmv = s1ps.tile([128, DG], F32, tag="psMV")
                    for kt in range(CT):
                        nc.tensor.matmul(psmv, lhsT=ms[:, kt, tt * 128:(tt + 1) * 128],
                                         rhs=wgs[:, kt, 2 * DG:3 * DG],
                                         start=(kt == 0), stop=(kt == CT - 1))
                    psmv_v = psmv.rearrange("p (h c) -> p h c", c=64)
                    nc.scalar.copy(v3v[:, tt, :, 0:64], psmv_v)
                    for xi in range(2):
                        ewv = ewsb[:, tt, xi * HPG:(xi + 1) * HPG]
                        nc.vector.tensor_tensor(
                            out=v3v[:, tt, :, 65 + 65 * xi:130 + 65 * xi],
                            in0=v3v[:, tt, :, 0:65],
                            in1=_ap(ewv, 0, [[1, HPG], [0, 65]]),
                            op=MULT)

            nc.sync.dma_start(wps, wpT.rearrange("(a p) c -> p a c", p=128))

            # ---------------- stage 2: attention ----------------
            with tc.tile_pool(name="epool", bufs=2) as epool, \
                 tc.tile_pool(name="est", bufs=2) as est, \
                 tc.tile_pool(name="sps", bufs=2, space="PSUM") as sps, \
                 tc.tile_pool(name="ups", bufs=1, space="PSUM") as ups:
                for h in range(HPG):
                    po = 64 * (h % 2)
                    pt = h // 2
                    E = epool.tile([128, TT, T], BF16, tag="E", name="E")
                    u = ups.tile([128, 2048], F32, tag="U", name="u")
                    for kb in range(TT):
                        q0 = kb * 128
                        kbs = slice(kb * 128, (kb + 1) * 128)
                        sx = sps.tile([128, 1024], F32, tag="SX", name="sx")
                        chunks = [(q0, 512), (512, 1024)] if q0 < 512 else [(q0, 1024)]
                        for c0, c1 in chunks:
                            nc.tensor.matmul(sx[:, c0:c1],
                                             lhsT=qk["ke"][po:po + 64, pt, kbs],
                                             rhs=qk["qe"][po:po + 64, pt, c0:c1],
                                             start=True, stop=True)
                        nc.scalar.activation(E[:, kb, q0:T], sx[:, q0:T], EXP)
                        nc.vector.tensor_tensor(out=E[:, kb, q0:q0 + 128],
                                                in0=E[:, kb, q0:q0 + 128],
                                                in1=msk, op=MULT)
                    for qb in range(TT):
                        for kp in range(qb + 1):
                            nc.tensor.matmul(
                                u[:, qb * 256:qb * 256 + VW],
                                lhsT=E[:, kp, qb * 128:qb * 128 + 128],
                                rhs=v3v[:, kp, h, :],
                                start=(kp == 0), stop=(kp == qb))
                    # epilogue: normalize 3 variants, min/max over interval pair
                    ru = est.tile([128, 24], F32, tag="ru")       # [qb, 3]
                    nc.vector.reciprocal(_ap(ru, 0, [[3, 8], [1, 3]]),
                                         _ap(u, 64, [[256, 8], [65, 3]]))
                    nc.vector.tensor_tensor(
                        out=ysl["ye"][:, :, h * 64:(h + 1) * 64],
                        in0=_ap(u, 0, [[256, 8], [1, 64]]),
                        in1=_ap(ru, 0, [[3, 8], [0, 64]]),
                        op=MULT)
                    sc = est.tile([128, 8, 2, 64], BF16, tag="sc")
                    nc.vector.tensor_tensor(
                        out=sc,
                        in0=_ap(u, 65, [[256, 8], [65, 2], [1, 64]]),
                        in1=_ap(ru, 1, [[3, 8], [1, 2], [0, 64]]),
                        op=MULT)
                    nc.vector.tensor_tensor(out=ysl["yl"][:, :, h * 64:(h + 1) * 64],
                                            in0=sc[:, :, 0, :], in1=sc[:, :, 1, :],
                                            op=MIN)
                    nc.vector.tensor_tensor(out=ysl["yu"][:, :, h * 64:(h + 1) * 64],
                                            in0=sc[:, :, 0, :], in1=sc[:, :, 1, :],
                                            op=MAX)

            # ---------------- stage 3: output projection ----------------
            with tc.tile_pool(name="s3ps", bufs=3, space="PSUM") as s3ps, \
                 tc.tile_pool(name="s3tp", bufs=2, space="PSUM") as s3tp, \
                 tc.tile_pool(name="s3sb", bufs=3) as s3sb, \
                 tc.tile_pool(name="yTp", bufs=1) as yTp:
                outs3 = (("ye", oy), ("yl", ol), ("yu", ou))
                yTs = {nm: yTp.tile([128, MT, T], BF16, tag="yT" + nm, name=nm)
                       for nm, _ in outs3}
                for nm, _ in outs3:
                    yT = yTs[nm]
                    for tt in range(TT):
                        pst = s3tp.tile([128, MT, 128], BF16, tag="pst")
                        for dt in range(MT):
                            nc.tensor.transpose(pst[:, dt, :],
                                                ysl[nm][:, tt, dt * 128:(dt + 1) * 128],
                                                ident)
                        if tt % 2:
                            nc.scalar.copy(_ap(yT, tt * 128, [[T, MT], [1, 128]]), pst)
                        else:
                            nc.vector.tensor_copy(_ap(yT, tt * 128, [[T, MT], [1, 128]]), pst)
                for tt in range(TT):
                    for oi, (nm, odram) in enumerate(outs3):
                        yT = yTs[nm]
                        ost = s3sb.tile([128, C], BF16, tag="ost")
                        ps = s3ps.tile([128, 1024], F32, tag="ps3")
                        for n0, nn in ((0, 512), (512, 256)):
                            for dt in range(MT):
                                nc.tensor.matmul(ps[:, n0:n0 + nn],
                                                 lhsT=yT[:, dt, tt * 128:(tt + 1) * 128],
                                                 rhs=wps[:, dt, n0:n0 + nn],
                                                 start=(dt == 0), stop=(dt == MT - 1))
                        if (tt + oi) % 2:
                            nc.scalar.copy(ost, ps[:, 0:C])
                        else:
                            nc.vector.tensor_copy(ost, ps[:, 0:C])
                        nc.sync.dma_start(odram[tt * 128:(tt + 1) * 128, :], ost)

    for _rep in range(reps):
        _once(_rep)


_NC_CACHE = {}


def _build_nc(reps=1):
    if reps not in _NC_CACHE:
        nc = bacc.Bacc("TRN2", target_bir_lowering=False, debug=False)
        with tile.TileContext(nc) as tc:
            _body(tc, reps)
        nc.compile()
        _NC_CACHE[reps] = nc
    return _NC_CACHE[reps]


def _host_consts(x_lower, x_upper, Wqkv):
    d = 0.5 * (np.asarray(x_upper, np.float64) - np.asarray(x_lower, np.float64))
    d0 = float(d.mean())
    W = np.asarray(Wqkv, np.float64)
    qd = d0 * np.abs(W[0:C]).sum(axis=1)          # [C]
    kd = d0 * np.abs(W[C:2 * C]).sum(axis=1)
    vd = d0 * np.abs(W[2 * C:3 * C]).sum(axis=1)
    return d, d0, qd, kd, vd


def _prep_inputs(x, x_lower, x_upper, Wqkv, Wproj):
    x64 = np.asarray(x, np.float64)
    W = np.asarray(Wqkv, np.float64)
    _, _, qd, _, _ = _host_consts(x_lower, x_upper, Wqkv)
    scale = 1.0 / np.sqrt(np.float64(D))
    # w[s,h] = qd_h . (Wk_h x_s) * scale = x_s . (Wk_h^T qd_h) * scale
    Wk = W[C:2 * C]
    Gm = np.zeros((C, H))
    for h in range(H):
        Gm[:, h] = Wk[h * D:(h + 1) * D].T @ qd[h * D:(h + 1) * D]
    w_all = np.einsum('btc,ch->bth', x64, Gm) * scale        # [B,T,H]
    ewm = np.exp(-w_all)
    ewp = np.exp(w_all)
    WqkvT = np.ascontiguousarray(Wqkv.T)          # [768, 2304]
    WprojT = np.ascontiguousarray(Wproj.T)        # [768, 768]
    in_maps = []
    for c in range(N_CORES):
        b, g = c // G, c % G
        sl = slice(g * DG, (g + 1) * DG)
        wg_g = np.concatenate([WqkvT[:, sl],
                               WqkvT[:, C + g * DG:C + (g + 1) * DG] * np.float32(scale),
                               WqkvT[:, 2 * C + g * DG:2 * C + (g + 1) * DG]], axis=1)
        ew_g = np.concatenate([ewm[b, :, g * HPG:(g + 1) * HPG],
                               ewp[b, :, g * HPG:(g + 1) * HPG]], axis=1)  # [T, 12]
        in_maps.append({
            "mT": np.ascontiguousarray(x[c // G].T).astype(bfloat16),
            "wg": wg_g.astype(bfloat16),
            "wpT": np.ascontiguousarray(WprojT[sl, :]).astype(bfloat16),
            "ew": np.ascontiguousarray(ew_g).astype(np.float32),
        })
    return in_maps


_RUNNER = {}


def _get_runner(reps=1):
    """Build (once) a cached sharded jit callable over the 8 cores."""
    if reps in _RUNNER:
        return _RUNNER[reps]
    import jax
    from jax.experimental.shard_map import shard_map
    from jax.sharding import Mesh, PartitionSpec
    from concourse import bass2jax as b2j
    from concourse import mybir as _mb

    nc = _build_nc(reps)
    b2j.install_neuronx_cc_hook()
    partition_name = nc.partition_id_tensor.name if nc.partition_id_tensor else None
    in_names, out_names, out_avals, zero_outs = [], [], [], []
    for alloc in nc.m.functions[0].allocations:
        if not isinstance(_mb.MemoryLocationSet, type) or not isinstance(alloc, _mb.MemoryLocationSet):
            continue
        name = alloc.memorylocations[0].name
        if alloc.kind == "ExternalInput":
            if name != partition_name:
                in_names.append(name)
        elif alloc.kind == "ExternalOutput":
            out_names.append(name)
            shape = tuple(alloc.tensor_shape)
            dtype = _mb.dt.np(alloc.dtype)
            out_avals.append(jax.core.ShapedArray(shape, dtype))
            zero_outs.append(np.zeros(shape, dtype))
    n_params = len(in_names)
    n_outs = len(out_avals)
    all_names = in_names + out_names
    if partition_name is not None:
        all_names = all_names + [partition_name]
    donate = tuple(range(n_params, n_params + n_outs))

    def _bodyfn(*args):
        operands = list(args)
        if partition_name is not None:
            operands.append(b2j.partition_id_tensor())
        outs = b2j._bass_exec_p.bind(
            *operands,
            out_avals=tuple(out_avals),
            in_names=tuple(all_names),
            out_names=tuple(out_names),
            lowering_input_output_aliases=(),
            sim_require_finite=True,
            sim_require_nnan=True,
            nc=nc,
        )
        return tuple(outs)

    devices = jax.devices()[:N_CORES]
    mesh = Mesh(np.asarray(devices), ("core",))
    in_specs = (PartitionSpec("core"),) * (n_params + n_outs)
    out_specs = (PartitionSpec("core"),) * n_outs
    sharded = jax.jit(
        shard_map(_bodyfn, mesh=mesh, in_specs=in_specs, out_specs=out_specs,
                  check_rep=False),
        donate_argnums=donate, keep_unused=True)
    _RUNNER[reps] = (sharded, in_names, out_names, out_avals, zero_outs)
    return _RUNNER[reps]


def _chain_runner(n_iter):
    """Jit that executes the kernel n_iter times back-to-back on device."""
    import jax
    from jax.experimental.shard_map import shard_map
    from jax.sharding import Mesh, PartitionSpec
    from concourse import bass2jax as b2j

    nc = _build_nc()
    sharded, in_names, out_names, out_avals, zero_outs = _get_runner()
    partition_name = nc.partition_id_tensor.name if nc.partition_id_tensor else None
    all_names = in_names + out_names
    if partition_name is not None:
        all_names = all_names + [partition_name]
    n_params = len(in_names)
    n_outs = len(out_avals)

    def _bodyfn(*args):
        operands = list(args)
        if partition_name is not None:
            operands.append(b2j.partition_id_tensor())
        return tuple(b2j._bass_exec_p.bind(
            *operands,
            out_avals=tuple(out_avals), in_names=tuple(all_names),
            out_names=tuple(out_names), lowering_input_output_aliases=(),
            sim_require_finite=True, sim_require_nnan=True, nc=nc))

    def f(*args):
        ins, outs = args[:n_params], args[n_params:]
        for _ in range(n_iter):
            outs = _bodyfn(*ins, *outs)
        return outs

    devices = __import__("jax").devices()[:N_CORES]
    mesh = Mesh(np.asarray(devices), ("core",))
    in_specs = (PartitionSpec("core"),) * (n_params + n_outs)
    out_specs = (PartitionSpec("core"),) * n_outs
    donate = tuple(range(n_params, n_params + n_outs))
    return jax.jit(shard_map(f, mesh=mesh, in_specs=in_specs, out_specs=out_specs,
                             check_rep=False),
                   donate_argnums=donate, keep_unused=True)


def _run(in_maps):
    sharded, in_names, out_names, out_avals, zero_outs = _get_runner()
    concat_in = [np.concatenate([in_maps[c][n] for c in range(N_CORES)], axis=0)
                 for n in in_names]
    concat_zeros = [np.zeros((N_CORES * z.shape[0], *z.shape[1:]), z.dtype)
                    for z in zero_outs]
    out_arrs = sharded(*concat_in, *concat_zeros)
    return [{n: np.asarray(out_arrs[i]).reshape(N_CORES, *out_avals[i].shape)[c]
             for i, n in enumerate(out_names)}
            for c in range(N_CORES)]


def _numpy_fallback(x, x_lower, x_upper, Wqkv, Wproj):
    """Exact fp64 host reference; used if inputs violate the constant-radius
    midpoint structure the device fast path assumes."""
    xf = x.astype(np.float64)
    W = Wqkv.astype(np.float64)
    Wp_ = Wproj.astype(np.float64)
    tril = np.tril(np.ones((T, T), bool))
    sc = 1.0 / np.sqrt(D)

    def heads(t):
        return t.reshape(B, T, H, D).transpose(0, 2, 1, 3)

    def probs(a, bb):
        s = np.einsum('bhtd,bhsd->bhts', a, bb) * sc
        s = np.where(tril, s, -np.inf)
        e = np.exp(s - s.max(-1, keepdims=True))
        return e / e.sum(-1, keepdims=True)

    q, k, v = (heads(t) for t in np.split(xf @ W.T, 3, axis=-1))
    Wpos = np.maximum(W, 0); Wneg = np.minimum(W, 0)
    lo = x_lower.astype(np.float64) @ Wpos.T + x_upper.astype(np.float64) @ Wneg.T
    hi = x_upper.astype(np.float64) @ Wpos.T + x_lower.astype(np.float64) @ Wneg.T
    ql, kl, vl = (heads(t) for t in np.split(lo, 3, axis=-1))
    qu, ku, vu = (heads(t) for t in np.split(hi, 3, axis=-1))
    y = np.einsum('bhts,bhsd->bhtd', probs(q, k), v)
    outs = []
    for (a, bb) in ((ql, kl), (ql, ku), (qu, kl), (qu, ku)):
        A = probs(a, bb)
        outs.append(np.einsum('bhts,bhsd->bhtd', A, vl))
        outs.append(np.einsum('bhts,bhsd->bhtd', A, vu))
    y_all = np.stack(outs)

    def merge(t):
        return t.transpose(0, 2, 1, 3).reshape(B, T, C)

    return (np.float32(merge(y) @ Wp_.T), np.float32(merge(y_all.min(0)) @ Wp_.T),
            np.float32(merge(y_all.max(0)) @ Wp_.T))


def kernel(x, x_lower, x_upper, Wqkv, Wproj):
    x = np.asarray(x)
    x_lower = np.asarray(x_lower)
    x_upper = np.asarray(x_upper)
    Wqkv = np.asarray(Wqkv)
    Wproj = np.asarray(Wproj)
    d, d0, qd, kd, vd = _host_consts(x_lower, x_upper, Wqkv)
    m_chk = 0.5 * (x_lower.astype(np.float64) + x_upper.astype(np.float64))
    if not (np.allclose(x.astype(np.float32), m_chk.astype(np.float32),
                        rtol=1e-5, atol=1e-6)
            and d0 >= 0.0 and np.abs(d - d0).max() < 1e-5):
        return _numpy_fallback(x, x_lower, x_upper, Wqkv, Wproj)
    in_maps = _prep_inputs(x, x_lower, x_upper, Wqkv, Wproj)
    res = _run(in_maps)
    cvec = (vd @ Wproj.T.astype(np.float64)).astype(np.float32)   # [C]
    y = np.zeros((B, T, C), np.float32)
    yl = np.zeros((B, T, C), np.float32)
    yu = np.zeros((B, T, C), np.float32)
    for c in range(N_CORES):
        b = c // G
        y[b] += res[c]["oy"].astype(np.float32)
        yl[b] += res[c]["ol"].astype(np.float32)
        yu[b] += res[c]["ou"].astype(np.float32)
    yl -= cvec[None, None, :]
    yu += cvec[None, None, :]
    return (y, yl, yu)
